# revision 13
# baseline (speedup 1.0000x reference)
"""Block-sparse attention (local + vertical-strided causal mask) on 8 TRN2 cores.

Sharding: one head per NeuronCore (H=8, n_cores=8).

Per-core device algorithm (head h, residue r = 7-h):
  The 4096x4096 score matrix is processed at 128x128 granularity:
  "pair" i = q block-rows (2i, 2i+1) (128 q tokens), "chunk" = 128 k tokens
  (2 mask blocks of 64). Local window -> chunks c in [i-8, i] of K itself;
  vertical-strided blocks -> host-gathered K_vert (6 blocks of 64, kb = 8j+r),
  processed as 3 chunks shared by all cores.

  Stripe-major schedule: pairs are processed in stripes of 3 (matching the
  3-pair oacc PSUM tiles); within a stripe, visits are grouped by k-chunk so
  consecutive S^T matmuls share the stationary kt chunk and batch into
  multi-pair (up to 384-col) matmuls with one LDWEIGHTS.

  S^T orientation: S^T[k,q] = kT_chunk.T @ qT_pair  (PE, bf16; sm_scale
  pre-folded into qT on host)
  P^T = exp(S^T)                                     (ACT, one call per group)
  masking is multiplicative post-exp: diag triangle (GPSIMD), window-start /
  vert-validity (DVE) via 9 shared [128,128] bf16 0/1 tiles
  oacc[q, 0:129] += P^T_chunk.T @ [V | 1]_chunk      (PE, PSUM-accumulated;
  PV matmuls of group g-2 are interleaved between the S^T runs of group g so
  their LDWEIGHTS hide under the long S^T streams)
  col 128 of oacc = softmax denominator; copied PSUM->SBUF (DVE) and stored
  unnormalized; the host divides by the denominator column.
"""

import numpy as np
import ml_dtypes

BF16 = ml_dtypes.bfloat16

H = 8
S = 4096
D = 128
BLK = 64
NB = S // BLK        # 64 block rows
NPAIR = NB // 2      # 32 row pairs
NVSLOT = 6           # usable vertical slots (kb = 8j + r <= 47)
NVC = NVSLOT // 2    # 3 vertical chunks
STRIPE = 3           # pairs per oacc tile
SSTRIPE = 6          # pairs per schedule stripe (S^T chunk-run length)
GROUP = 12           # PSUM staging slots per exp group (12 * 128 f32 = 3 banks)
NMTILE = 9           # shared multiplicative mask tiles (window-start + 8 vert)


def make_schedule():
    """Global ordered visit list. visit = (kind, idx, pair)
    kind "local": idx = chunk c (k blocks 2c, 2c+1)
    kind "vert":  idx = vc (K_vert slots 2vc, 2vc+1; pair i gets vc iff
    8*vc+8 <= i, i.e. the vert chunk lies fully before the local window)

    Stripe-major order: pairs in stripes of SSTRIPE; within a stripe visits
    are grouped by chunk (verts first, then locals old to new), pairs
    ascending within a chunk -- so consecutive slots share the stationary
    kt chunk and batch into multi-pair S^T matmuls (fewer LDWEIGHTS)."""
    visits = []
    for t in range((NPAIR + SSTRIPE - 1) // SSTRIPE):
        pairs = list(range(SSTRIPE * t, min(SSTRIPE * t + SSTRIPE, NPAIR)))
        for vc in range(NVC):
            visits.extend(("vert", vc, i) for i in pairs if i >= 8 * vc + 8)
        for c in range(max(0, SSTRIPE * t - 8), SSTRIPE * t + SSTRIPE):
            visits.extend(("local", c, i) for i in pairs if c <= i <= c + 8)
    return visits


def mask_tile_idx(kind, idx, i):
    """Multiplicative mask tile for a visit (None = fully valid).
    tile 0: window-start (local chunk c == i-8): only (k>=64, q<64) valid.
    tiles 1..8: vert validity for m = i - 8*vc in [8, 15] (for m >= 16
    every head's slots are fully valid -> no mask)."""
    if kind == "local" and idx == i - 8:
        return 0
    if kind == "vert" and i < 8 * idx + 16:
        return 1 + (i - 8 * idx - 8)
    return None


_PROGRAMS = {}


def _build_program(sm_scale, pv_delay=2, group=GROUP, stage_bufs=2, pt_bufs=4,
                   ob_bufs=3, oacc_bufs=2, n_warm=6):
    import concourse.bass as bass
    import concourse.mybir as mybir
    import concourse.tile as tile
    from concourse import bacc

    fp32 = mybir.dt.float32
    bf16 = mybir.dt.bfloat16

    nc = bacc.Bacc("TRN2", target_bir_lowering=False, debug=False, num_devices=H)

    qt_d = nc.dram_tensor("qt", [D, S], bf16, kind="ExternalInput").ap()
    kt_d = nc.dram_tensor("kt", [D, S], bf16, kind="ExternalInput").ap()
    ktv_d = nc.dram_tensor("ktv", [D, NVSLOT * BLK], bf16, kind="ExternalInput").ap()
    vvaug_d = nc.dram_tensor("vvaug", [128, NVC * (D + 1)], bf16,
                             kind="ExternalInput").ap()
    vaug_d = nc.dram_tensor("vaug", [128, NPAIR, D + 1], bf16, kind="ExternalInput").ap()
    masks_d = nc.dram_tensor("masks", [128, NMTILE * 128], bf16,
                             kind="ExternalInput").ap()
    tri_d = nc.dram_tensor("tri", [128, 128], bf16, kind="ExternalInput").ap()
    o_d = nc.dram_tensor("o", [128, NPAIR, D + 1], fp32, kind="ExternalOutput").ap()

    visits = make_schedule()
    first = {}
    last = {}
    for g, (kind, idx, i) in enumerate(visits):
        first.setdefault(i, g)
        last[i] = g
    # PSUM start_tensor_calc zeroes the full 2KB bank (zero-region), so only
    # the first matmul touching an oacc tile may carry start=True.
    tile_first = {}
    for g, (kind, idx, i) in enumerate(visits):
        tile_first.setdefault(i // STRIPE, g)

    with tile.TileContext(nc) as tc:
        with (
            tc.tile_pool(name="big", bufs=1) as big,
            tc.tile_pool(name="stage", bufs=stage_bufs, space="PSUM") as stagep,
            tc.tile_pool(name="oacc", bufs=oacc_bufs, space="PSUM") as oaccp,
            tc.tile_pool(name="pt", bufs=pt_bufs) as ptp,
            tc.tile_pool(name="ob", bufs=ob_bufs) as obp,
        ):
            _emit_body(nc, tc, locals(), sm_scale, pv_delay=pv_delay, group=group,
                       n_warm=n_warm)
    nc.compile()
    return nc


def _emit_body(nc, tc, env, sm_scale, pv_delay=2, group=GROUP, n_warm=6):
    GROUP = group
    import concourse.mybir as mybir

    fp32 = mybir.dt.float32
    bf16 = mybir.dt.bfloat16
    big, stagep, oaccp, ptp, obp = (
        env["big"], env["stagep"], env["oaccp"], env["ptp"], env["obp"]
    )
    qt_d, kt_d, ktv_d, vvaug_d, vaug_d, masks_d, tri_d, o_d = (
        env["qt_d"], env["kt_d"], env["ktv_d"], env["vvaug_d"], env["vaug_d"],
        env["masks_d"], env["tri_d"], env["o_d"],
    )
    visits, first, last, tile_first = (
        env["visits"], env["first"], env["last"], env["tile_first"],
    )
    n_groups = (len(visits) + GROUP - 1) // GROUP

    qt = big.tile([D, S], bf16)
    kt = big.tile([D, S], bf16)
    ktv = big.tile([D, NVSLOT * BLK], bf16)
    vvaug = big.tile([128, NVC * (D + 1)], bf16)
    vaug = big.tile([128, NPAIR, D + 1], bf16)
    masks = big.tile([128, NMTILE * 128], bf16)
    tri = big.tile([128, 128], bf16)
    wtile = big.tile([128, 512], bf16)
    tld = big.tile([128, 1], fp32)

    # Preload the exp spline table-set at t=0: memzero + dummy 1-col exp on
    # the scalar queue only, so the ~2.7us ACT_TABLE_LOAD+DRAIN overlaps the
    # initial input DMA wait instead of serializing before the first group.
    nc.scalar.memzero(tld[:])
    nc.scalar.activation(out=tld[:], in_=tld[:],
                         func=mybir.ActivationFunctionType.Exp)

    # Inputs spread across the three DMA queues (SP-HW, ACT-HW, Pool-SW),
    # first-needed-first so the leading stripes' data lands fast. The
    # scalar HW queue issues nothing before the dummy exp, so the exp
    # table preload starts immediately; qt after the first slice rides
    # behind it (needed only from group ~2 on).
    nc.sync.dma_start(out=kt[:, 0:768], in_=kt_d[:, 0:768])
    nc.sync.dma_start(out=qt[:, 0:768], in_=qt_d[:, 0:768])
    nc.sync.dma_start(out=tri[:], in_=tri_d[:])
    nc.sync.dma_start(out=masks[:], in_=masks_d[:])
    nc.scalar.dma_start(out=qt[:, 768:1792], in_=qt_d[:, 768:1792])
    nc.sync.dma_start(out=kt[:, 768:1792], in_=kt_d[:, 768:1792])
    nc.sync.dma_start(out=ktv[:], in_=ktv_d[:])
    nc.scalar.dma_start(out=qt[:, 1792:3072], in_=qt_d[:, 1792:3072])
    nc.sync.dma_start(out=kt[:, 1792:3072], in_=kt_d[:, 1792:3072])
    nc.sync.dma_start(out=kt[:, 3072:4096], in_=kt_d[:, 3072:4096])
    nc.scalar.dma_start(out=qt[:, 3072:4096], in_=qt_d[:, 3072:4096])
    nc.scalar.dma_start(out=vvaug[:], in_=vvaug_d[:])
    nc.gpsimd.dma_start(out=vaug[:, 0:6], in_=vaug_d[:, 0:6])
    nc.gpsimd.dma_start(out=vaug[:, 6:18], in_=vaug_d[:, 6:18])
    nc.gpsimd.dma_start(out=vaug[:, 18:32], in_=vaug_d[:, 18:32])

    nc.vector.memset(wtile[:], 0.0)

    # PE p-state warmup: stream throwaway matmuls on a memset tile so the
    # array is ramping while the first inputs arrive.
    warm = stagep.tile([128, GROUP * 128], fp32, tag="stage")
    for w in range(n_warm):
        nc.tensor.matmul(
            warm[:, (w % 3) * 512 : (w % 3 + 1) * 512],
            wtile[:, 0:128],
            wtile[:, 0:512],
            start=True,
            stop=True,
            skip_group_check=True,
        )

    oacc_tiles = {}  # stripe (i//STRIPE) -> psum tile [128, STRIPE, 129]
    ready_pv = []    # PV slot closures whose exp+masks are complete
    later_pv = []    # (group, [closures]) not yet released

    def make_pv(g, kind, idx, i, ptt, s):
        def emit_pv():
            pg = i // STRIPE
            if pg not in oacc_tiles:
                oacc_tiles[pg] = oaccp.tile(
                    [128, STRIPE, D + 1], fp32, tag="oacc", name=f"oacc{pg}"
                )
            oacc = oacc_tiles[pg]
            if kind == "local":
                rhs = vaug[:, idx]
            else:
                rhs = vvaug[:, idx * (D + 1) : (idx + 1) * (D + 1)]
            nc.tensor.matmul(
                oacc[:, i % STRIPE],
                ptt[:, s * 128 : (s + 1) * 128],
                rhs,
                start=(g == tile_first[pg]),
                stop=(g == last[i]),
                skip_group_check=True,
            )
            # epilogue once per oacc tile (after its last pair closes): one
            # DVE read of the PSUM bank into SBUF, then an unnormalized
            # store (host divides by col 128).
            pg_pairs = [p for p in range(STRIPE * pg, STRIPE * (pg + 1))
                        if p < NPAIR]
            if i == pg_pairs[-1] and g == last[i]:
                npp = len(pg_pairs)
                osb = obp.tile([128, STRIPE, D + 1], fp32, tag="osb")
                nc.vector.tensor_copy(osb[:, 0:npp], oacc[:, 0:npp])
                nc.sync.dma_start(
                    out=o_d[:, STRIPE * pg : STRIPE * pg + npp, :],
                    in_=osb[:, 0:npp],
                )
        return emit_pv

    for gi in range(n_groups):
        gvis = visits[gi * GROUP : (gi + 1) * GROUP]
        n = len(gvis)
        stage = stagep.tile([128, GROUP * 128], fp32, tag="stage")
        ptt = ptp.tile([128, GROUP * 128], bf16, tag="pt")

        # release PV groups whose exp+masks are at least pv_delay behind
        while later_pv and later_pv[0][0] <= gi - pv_delay:
            ready_pv.extend(later_pv.pop(0)[1])

        # --- S^T matmuls, batched over runs of consecutive pairs sharing
        # one k-chunk, split at 4-slot (one PSUM bank) bounds; start=True
        # only on the first run per bank (bank zero-region). PV matmuls of
        # older groups are interleaved between runs so their LDWEIGHTS
        # hide under the long S^T streams.
        runs = []
        s = 0
        while s < n:
            kind, idx, i0 = gvis[s]
            e = s + 1
            while (
                e < n
                and e % 4 != 0
                and gvis[e][0] == kind
                and gvis[e][1] == idx
                and gvis[e][2] == gvis[e - 1][2] + 1
            ):
                e += 1
            runs.append((s, e, kind, idx, i0))
            s = e

        npv = len(ready_pv)
        seen_banks = set()
        for ri, (s, e, kind, idx, i0) in enumerate(runs):
            lhsT = (
                kt[:, idx * 128 : (idx + 1) * 128]
                if kind == "local"
                else ktv[:, idx * 128 : (idx + 1) * 128]
            )
            bank = s // 4
            nc.tensor.matmul(
                stage[:, s * 128 : e * 128],
                lhsT,
                qt[:, i0 * 128 : (i0 + (e - s)) * 128],
                start=bank not in seen_banks,
                stop=True,
                skip_group_check=True,
            )
            seen_banks.add(bank)
            # interleave a fair share of pending PV matmuls after this run
            want = (npv * (ri + 1)) // len(runs)
            while npv - len(ready_pv) < want:
                ready_pv.pop(0)()

        # --- exp for the group (sm_scale pre-folded into qt on host)
        nc.scalar.activation(
            out=ptt[:, 0 : n * 128],
            in_=stage[:, 0 : n * 128],
            func=mybir.ActivationFunctionType.Exp,
        )

        # --- multiplicative masks on P^T (DVE): diag triangle,
        # window-start, vert-validity
        pv_slots = []
        for s, (kind, idx, i) in enumerate(gvis):
            sl = slice(s * 128, (s + 1) * 128)
            if kind == "local" and idx == i:
                nc.vector.tensor_mul(ptt[:, sl], ptt[:, sl], tri[:])
            else:
                mi = mask_tile_idx(kind, idx, i)
                if mi is not None:
                    nc.vector.tensor_mul(
                        ptt[:, sl], ptt[:, sl],
                        masks[:, mi * 128 : (mi + 1) * 128],
                    )
            pv_slots.append(make_pv(gi * GROUP + s, kind, idx, i, ptt, s))

        later_pv.append((gi, pv_slots))

    for _, slots in later_pv:
        for f in slots:
            f()
    for f in ready_pv:
        f()


def _get_program(smv=0.08838834764831845):
    key = float(smv)
    if key not in _PROGRAMS:
        _PROGRAMS[key] = _build_program(key)
    return _PROGRAMS[key]


def _host_inputs(q, k, v, sm_scale):
    """Per-core input dicts (host-side shard + layout)."""
    q = np.asarray(q, dtype=np.float32)
    k = np.asarray(k, dtype=np.float32)
    v = np.asarray(v, dtype=np.float32)
    smv = float(np.asarray(sm_scale, dtype=np.float32))

    p = np.arange(128)
    j = np.arange(128)
    tri = np.zeros((128, 128), dtype=BF16)
    tri[p[:, None] <= p[None, :]] = BF16(1.0)

    ins = []
    for h in range(H):
        r = 7 - h
        qh, kh, vh = q[0, h], k[0, h], v[0, h]
        qt = np.ascontiguousarray((qh * smv).T).astype(BF16)
        kt = np.ascontiguousarray(kh.T).astype(BF16)
        vblocks = [8 * j_ + r for j_ in range(NVSLOT)]
        kv = np.concatenate([kh[b * BLK : (b + 1) * BLK] for b in vblocks], axis=0)
        ktv = np.ascontiguousarray(kv.T).astype(BF16)  # [128, 384]
        vaug = np.concatenate(
            [vh, np.ones((S, 1), np.float32)], axis=1
        ).astype(BF16)  # [4096, 129]
        vaug = np.ascontiguousarray(
            vaug.reshape(NPAIR, 128, D + 1).transpose(1, 0, 2)
        )  # [128, 32, 129]
        vv = np.concatenate([vh[b * BLK : (b + 1) * BLK] for b in vblocks], axis=0)
        vvaug = np.concatenate([vv, np.ones((NVSLOT * BLK, 1), np.float32)], axis=1)
        vvaug = np.ascontiguousarray(
            vvaug.astype(BF16).reshape(NVC, 128, D + 1).transpose(1, 0, 2)
        ).reshape(128, NVC * (D + 1))  # [128, 387]

        # Shared multiplicative 0/1 mask tiles [128 k, 128 q]:
        # tile 0 (window-start, chunk c = i-8): valid iff (k>=64 and q<64)
        # tiles 1..8 (vert validity, m = i-8vc in 8..15): k half h has
        # kb = 16vc + 8h + r; valid iff kb <= 2i-16 (q<64) / 2i-15 (q>=64),
        # i.e. 8h + r <= 2m-16 / 2m-15.
        masks = np.zeros((128, NMTILE * 128), dtype=BF16)
        masks[:, 0:128] = (
            (p[:, None] >= 64) & (j[None, :] < 64)
        ).astype(np.float32).astype(BF16)
        hrow = 8 * (p >= 64).astype(np.int64) + r        # 8h + r per k row
        for m in range(8, 16):
            thr = np.where(j < 64, 2 * m - 16, 2 * m - 15)
            tilem = (hrow[:, None] <= thr[None, :]).astype(np.float32)
            masks[:, (m - 7) * 128 : (m - 6) * 128] = tilem.astype(BF16)

        ins.append(dict(qt=qt, kt=kt, ktv=ktv, vvaug=vvaug, vaug=vaug,
                        masks=masks, tri=tri))
    return ins


def kernel(q, k, v, sm_scale):
    from concourse.bass_utils import run_bass_kernel_spmd

    smv = float(np.asarray(sm_scale, dtype=np.float32))
    nc = _get_program(smv)
    ins = _host_inputs(q, k, v, sm_scale)
    res = run_bass_kernel_spmd(nc, ins, core_ids=list(range(H)))
    outs = []
    for h in range(H):
        o = res.results[h]["o"]  # [128, NPAIR, 129]
        o = o.transpose(1, 0, 2).reshape(S, D + 1)
        outs.append(o[:, :D] / o[:, D : D + 1])
    out = np.stack(outs, axis=0)[None]
    return out.astype(np.float32)
